# revision 3
# baseline (speedup 1.0000x reference)
"""Trainium2 Bass kernel for nn_AttentionLayer. v5 (~214 us, from 435 us v4)

Data-parallel over batch: one [256,4096] attention problem per NeuronCore.

Per-core architecture (per 512-col j-chunk, per pair of 128-row i-tiles):
  - score pair: two K=64 bf16 matmuls into one 2-bank PSUM pair tile at
    PE row-groups (0,0)/(64,0); issued back to back they execute
    CONCURRENTLY on the PE (row tiling, measured dt ~3ns).
  - one exp per pair over the whole [128,1024] 2-bank PSUM tile:
    - most pairs: ACT exp writing fp8e4 directly (scale=0.125)
    - DVE_PAIRS: single-instruction Schraudolph exp on DVE: tensor_scalar
      (beta*1.4427 + 56) with int8 round-nearest convert whose bit pattern
      IS the fp8e4m3 of exp(beta/8); softmax ratios cancel most of the
      ~4 percent approx error.
  - PV and den in fp8 DoubleRow (contraction 256 per matmul): per pair
    2 PV matmuls [128,2,128]x[128,2,512] and 1 den matmul with a tiny
    [128,2,1] ones stationary (den rides at ~215ns per pair).
  - den reciprocal: [1,512] remapped to [128,4] by SBUF-SBUF DMA so the
    DVE reciprocal costs ~180ns instead of 3.3us; broadcast back via a
    rank-1 PE matmul; epilogue multiplies read that PSUM directly.
  - small loads ride the idle GPSIMD DMA queue; relus are hoisted two
    chunks ahead of the QKV compute; chunk 6's QKV PSUM and chunk 7's
    four psv tiles borrow the den+misc banks, so both the psb pair
    rotation and the oacc banks drain early and jc0's scores AND PV
    accumulation overlap the prologue tail.
  - _prune_redundant_waits removes semaphore waits provably satisfied at
    dispatch (monotone subsumption + self-wait margins): in steady state
    the PE carries exactly one wait per pair (the true exp dependency).
"""

import numpy as np

import bass_rust
import concourse.bass as bass
import concourse.tile as tile
from concourse import mybir
from concourse.bass_utils import run_bass_kernel_spmd

N_CORES = 8
C = 256
M = 64
HW = 4096
JC = 512
N_JC = HW // JC
N_IT = HW // 128
N_PAIR = N_IT // 2

F32 = mybir.dt.float32
F32R = mybir.dt.float32r
BF16 = mybir.dt.bfloat16
F8 = mybir.dt.float8e4
I8 = mybir.dt.int8

# Schraudolph exp -> fp8e4m3 bits: bits = round(beta_psum * A + 56).
# Full-128-row scores with duplicated q/k compute 2*beta, so the exp scale
# is 1/16 instead of 1/8.
SCH_A = float(0.0625 * 8.0 / np.log(2.0))
SCH_B = 56.0

# j-columns 0:EXP_SPLIT of each pair's exp run on ACT (true exp), the rest
# on DVE (Schraudolph). Splitting by j keeps every softmax column on a
# single engine so the approximation bias cancels in the P = E/den ratio.
EXP_SPLIT = 256

DR = mybir.MatmulPerfMode.DoubleRow


def _install_tile_drain_fix():
    def _drain_and_barrier(self, tick_clock, wait_clock):
        from concourse.tile import ScopedClock

        nc = self.nc
        probe = nc.sync.nop()
        wait_clock.add_sem_waits(
            probe.ins, ScopedClock({None: tick_clock.global_clock})
        )
        si = probe.ins.sync_info
        waits = list(si.on_wait) if si is not None else []
        probe.ins.sync_info = bass_rust.SyncInfo(on_wait=waits[:1], on_update=[])
        for w in waits[1:]:
            n = nc.sync.nop()
            n.ins.sync_info = bass_rust.SyncInfo(on_wait=[w], on_update=[])
        nc.sync.drain()
        nc.all_engine_barrier()
        assert self.sems is not None
        popped = nc._tile_sem_poison_stack.pop()
        assert popped is self._sem_poison
        nc.clear_and_free_semaphores(list(self.sems.allocated().values()))
        nc.all_engine_barrier()

    tile.TileContext._drain_and_barrier = _drain_and_barrier


def _split_multi_waits(nc):
    """walrus in this toolchain encodes at most one sync wait per
    instruction. Split any instruction carrying more onto single-wait
    NOPs inserted immediately before it on the same engine."""
    ctr = [0]

    def mk_nop(engine, wait):
        ctr[0] += 1
        n = mybir.InstNoOp(name=f"I-wsplit{ctr[0]}", ins=[], outs=[])
        n.engine = engine
        n.sync_info = bass_rust.SyncInfo(on_wait=[wait], on_update=[])
        return n

    for f in nc.m.functions:
        for bb in f.blocks:
            out = []
            changed = False
            for inst in bb.instructions:
                si = inst.sync_info
                waits = list(si.on_wait) if si is not None else []
                if len(waits) > 1:
                    for w in waits[:-1]:
                        out.append(mk_nop(inst.engine, w))
                    inst.sync_info = bass_rust.SyncInfo(
                        on_wait=[waits[-1]], on_update=list(si.on_update)
                    )
                    changed = True
                out.append(inst)
            if changed:
                bb.instructions = out


def _prune_redundant_waits(nc):
    """Remove semaphore waits that are provably satisfied at dispatch:

    1) same-stream monotone subsumption: an earlier instruction on the same
       engine already waited for sem >= v' with v' >= v (sems only count up);
    2) self-waits: a wait on the engine's OWN completion-count semaphore with
       a value far enough behind this instruction's position. For serial
       engines (ACT/DVE) completion of instr k-1 precedes start of k; for
       the PE (pipelined matmuls) a margin of 8 instructions covers the
       stream+drain overlap window.

    Each pruned wait removes a dispatch pipeline-break (~100-170ns) on an
    in-order engine.
    """
    from collections import defaultdict
    from concourse import mybir as mb

    PE = mybir.EngineType.PE
    SELF_OK = {PE, mybir.EngineType.DVE, mybir.EngineType.Activation}
    for f in nc.m.functions:
        for bb in f.blocks:
            upd_engines = defaultdict(set)
            bad_sems = set()
            for inst in bb.instructions:
                si = inst.sync_info
                if si is None:
                    continue
                for u in si.on_update:
                    if u.sync_type == "semaphore":
                        if u.update_mode == "sem-inc":
                            upd_engines[u.id].add(inst.engine)
                        else:
                            bad_sems.add(u.id)
            streams = defaultdict(list)
            for inst in bb.instructions:
                streams[inst.engine].append(inst)
            pruned = 0
            for eng, insts in streams.items():
                inc_count = defaultdict(int)
                max_waited = defaultdict(int)
                for inst in insts:
                    si = inst.sync_info
                    if si is not None and si.on_wait:
                        keep = []
                        for w in si.on_wait:
                            drop = False
                            if (
                                w.sync_type == "semaphore"
                                and w.wait_mode == "sem-ge-imm"
                                and w.id not in bad_sems
                            ):
                                v = w.wait_value
                                if v <= max_waited[w.id]:
                                    drop = True
                                elif (
                                    eng in SELF_OK
                                    and upd_engines.get(w.id) == {eng}
                                ):
                                    margin = 8 if eng == PE else 1
                                    if v <= inc_count[w.id] - margin:
                                        drop = True
                                if drop:
                                    pruned += 1
                                    max_waited[w.id] = max(max_waited[w.id], v)
                                else:
                                    keep.append(w)
                                    max_waited[w.id] = max(max_waited[w.id], v)
                            else:
                                keep.append(w)
                        if pruned and len(keep) != len(si.on_wait):
                            inst.sync_info = bass_rust.SyncInfo(
                                on_wait=keep, on_update=list(si.on_update)
                            )
                    if si is not None:
                        for u in si.on_update:
                            if (
                                u.sync_type == "semaphore"
                                and u.update_mode == "sem-inc"
                            ):
                                inc_count[u.id] += u.update_value
    return nc


def build(split_waits=True):
    _install_tile_drain_fix()
    nc = bass.Bass("TRN2", target_bir_lowering=False, debug=False)

    x_ext = nc.declare_dram_parameter("x", [C, HW], F32, isOutput=False)
    w_ext = nc.declare_dram_parameter("Wqkv", [2 * M + C, C], F32, isOutput=False)
    b_ext = nc.declare_dram_parameter("bqkv", [2 * M + C, 1], F32, isOutput=False)
    g_ext = nc.declare_dram_parameter("gamma", [1, 1], F32, isOutput=False)
    out_ext = nc.declare_dram_parameter("out", [C, HW], F32, isOutput=True)

    ident_dram = nc.inline_tensor(np.eye(128, dtype=np.float32), "ident128")
    onesr_dram = nc.inline_tensor(np.ones((1, 128), dtype=np.float32), "onesrow")

    with tile.TileContext(nc) as tc:
        with (
            tc.tile_pool(name="const", bufs=1) as constp,
            tc.tile_pool(name="xin", bufs=1) as xp,
            tc.tile_pool(name="xr", bufs=1) as xrp,
            tc.tile_pool(name="wld", bufs=1) as wldp,
            tc.tile_pool(name="wt", bufs=1) as wtp,
            tc.tile_pool(name="qk", bufs=1) as qkp,
            tc.tile_pool(name="vt", bufs=1) as vtp,
            tc.tile_pool(name="e", bufs=4) as ep,
            tc.tile_pool(name="osb", bufs=3) as osbp,
            tc.tile_pool(name="misc", bufs=1) as miscp,
            tc.tile_pool(name="ps_b", bufs=2, space="PSUM") as psb,
            tc.tile_pool(name="ps_acc", bufs=1, space="PSUM") as psacc,
            tc.tile_pool(name="ps_den", bufs=1, space="PSUM") as psden,
            tc.tile_pool(name="ps_misc", bufs=1, space="PSUM") as psmisc,
        ):
            # ---- constants + small loads on the idle GPSIMD dma queue so
            # neither the sync queue (x chunks) nor the ACT engine stalls ----
            ident = constp.tile([128, 128], F32)
            nc.gpsimd.dma_start(ident[:], ident_dram.ap()[:, :])
            w_rows = []
            for oc in range(3):
                wt_ = wldp.tile([128, C], F32, tag=f"wrows{oc}", name=f"wrows{oc}")
                nc.gpsimd.dma_start(wt_[:], w_ext.ap()[128 * oc : 128 * (oc + 1), :])
                w_rows.append(wt_)
            bias_qq = miscp.tile([128, 1], F32, tag="bqq")
            nc.gpsimd.dma_start(bias_qq[0:64, :], b_ext.ap()[0:64, :])
            nc.gpsimd.dma_start(bias_qq[64:128, :], b_ext.ap()[0:64, :])
            bias_kk = miscp.tile([128, 1], F32, tag="bkk")
            nc.gpsimd.dma_start(bias_kk[0:64, :], b_ext.ap()[64:128, :])
            nc.gpsimd.dma_start(bias_kk[64:128, :], b_ext.ap()[64:128, :])
            ones_row = constp.tile([1, 128], F32)
            nc.gpsimd.dma_start(ones_row[:], onesr_dram.ap()[:, :])
            gamma_sb = miscp.tile([1, 1], F32, tag="gam")
            nc.gpsimd.dma_start(gamma_sb[:], g_ext.ap()[:, :])
            bias_v = []
            for cc in range(2):
                bv = miscp.tile([128, 1], F32, tag=f"bv{cc}", name=f"bv{cc}")
                nc.gpsimd.dma_start(
                    bv[:], b_ext.ap()[128 + 128 * cc : 128 + 128 * (cc + 1), :]
                )
                bias_v.append(bv)
            ones_row_bf = constp.tile([1, 128], BF16)
            nc.vector.tensor_copy(ones_row_bf[:], ones_row[:])
            ones_f8 = constp.tile([128, 2, 16], F8)
            nc.vector.memset(ones_f8[:, :, :], 1.0)

            # ---- transpose W via PE (q/k first: they gate chunk 0 and do
            # not need gamma) ----
            wqqT = []
            wkkT = []
            for cc in range(2):
                ps = psmisc.tile([128, 512], F32, tag="psm")
                nc.tensor.transpose(
                    ps[:, 0:128], w_rows[0][:, 128 * cc : 128 * (cc + 1)], ident[:]
                )
                tq = wtp.tile([128, 128], BF16, tag=f"wqqT{cc}", name=f"wqqT{cc}")
                nc.vector.tensor_copy(tq[:, 0:64], ps[:, 0:64])
                nc.vector.tensor_copy(tq[:, 64:128], ps[:, 0:64])
                wqqT.append(tq)
                tk = wtp.tile([128, 128], BF16, tag=f"wkkT{cc}", name=f"wkkT{cc}")
                nc.vector.tensor_copy(tk[:, 0:64], ps[:, 64:128])
                nc.vector.tensor_copy(tk[:, 64:128], ps[:, 64:128])
                wkkT.append(tk)

            gamma_bf = miscp.tile([1, 1], BF16, tag="gambf")
            nc.vector.tensor_copy(gamma_bf[:], gamma_sb[:])
            g_ps = psmisc.tile([128, 512], F32, tag="psm")
            nc.tensor.matmul(
                g_ps[:, 0:1], ones_row_bf[:], gamma_bf[:], start=True, stop=True
            )
            gamma_bc = miscp.tile([128, 1], F32, tag="gbc_sb")
            nc.vector.tensor_copy(gamma_bc[:], g_ps[:, 0:1])
            gbv = []
            for cc in range(2):
                t = miscp.tile([128, 1], F32, tag=f"gbv{cc}", name=f"gbv{cc}")
                nc.vector.tensor_mul(t[:], bias_v[cc][:], gamma_bc[:])
                gbv.append(t)
            wvT = []
            for cc in range(2):
                t = wtp.tile([128, 256], BF16, tag=f"wvT{cc}", name=f"wvT{cc}")
                for oc in range(2):
                    ps = psmisc.tile([128, 512], F32, tag="psm")
                    nc.tensor.transpose(
                        ps[:, 0:128],
                        w_rows[1 + oc][:, 128 * cc : 128 * (cc + 1)],
                        ident[:],
                    )
                    # fold gamma into the v weights so the per-i-tile vtg
                    # copies don't need a scale
                    nc.vector.tensor_scalar_mul(
                        t[:, 128 * oc : 128 * (oc + 1)], ps[:, 0:128], gamma_bc[:]
                    )
                wvT.append(t)

            # ---- tiles ----
            x_sb = [
                xp.tile([128, HW], F32, tag=f"x{cc}", name=f"xchunk{cc}")
                for cc in range(2)
            ]
            xr_sb = xrp.tile([128, 2, HW], BF16, tag="xr", name="xr")
            qq_sb = qkp.tile([128, HW], BF16, tag="qq")
            kk_sb = qkp.tile([128, HW], BF16, tag="kk")
            vtg = vtp.tile([128, N_IT, 256], F8, tag="vtg")

            def emit_xdma(n):
                sl = slice(JC * n, JC * (n + 1))
                for cc in range(2):
                    nc.sync.dma_start(
                        x_sb[cc][:, sl], x_ext.ap()[128 * cc : 128 * (cc + 1), sl]
                    )

            def emit_relu(n):
                # relu + bf16 cast split across ACT and DVE
                sl = slice(JC * n, JC * (n + 1))
                nc.scalar.activation(
                    xr_sb[:, 0, sl], x_sb[0][:, sl],
                    mybir.ActivationFunctionType.Relu,
                )
                nc.vector.tensor_scalar_max(xr_sb[:, 1, sl], x_sb[1][:, sl], 0.0)

            def emit_chunk(n):
                sl = slice(JC * n, JC * (n + 1))
                if n == 6:
                    # park chunk 6's qk psum in the den+misc banks so the
                    # psb pair rotation drains two chunks earlier and the
                    # first attention pairs can overlap the prologue tail
                    qps = psden.tile([128, 512], F32, tag="den", name="den")
                    kps = psmisc.tile([128, 512], F32, tag="psm", name="kps")
                else:
                    pair = psb.tile([128, 2, 512], F32, tag="beta", name="qkps")
                    qps, kps = pair[:, 0, :], pair[:, 1, :]
                for kc in range(2):
                    nc.tensor.matmul(
                        qps, wqqT[kc][:], xr_sb[:, kc, sl],
                        start=(kc == 0), stop=(kc == 1),
                    )
                for kc in range(2):
                    nc.tensor.matmul(
                        kps, wkkT[kc][:], xr_sb[:, kc, sl],
                        start=(kc == 0), stop=(kc == 1),
                    )
                nc.vector.tensor_scalar_add(qq_sb[:, sl], qps, bias_qq[:])
                nc.vector.tensor_scalar_add(kk_sb[:, sl], kps, bias_kk[:])
                # v^T for the 4 i-tiles of this chunk (alternate oacc banks;
                # the LAST chunk's psv borrow the den+misc banks instead so
                # oacc frees a chunk early and jc0's PV accumulation can
                # overlap the prologue tail)
                for tt in range(4 * n, 4 * (n + 1)):
                    if tt >= 28:
                        if tt % 2 == 0:
                            psv = psden.tile([128, 512], F32, tag="den", name="den")
                        else:
                            psv = psmisc.tile([128, 512], F32, tag="psm", name="kps")
                    else:
                        psv = psacc.tile(
                            [128, 512], F32, tag=f"oacc{tt % 2}", name=f"oacc{tt % 2}"
                        )
                    for kc in range(2):
                        nc.tensor.matmul(
                            psv[:, 0:256],
                            xr_sb[:, kc, 128 * tt : 128 * (tt + 1)],
                            wvT[kc][:],
                            start=(kc == 0), stop=(kc == 1),
                        )
                    if tt % 2 == 0:
                        nc.scalar.copy(vtg[:, tt, :], psv[:, 0:256])
                    else:
                        nc.vector.tensor_copy(vtg[:, tt, :], psv[:, 0:256])

            def emit_pair(jc, p, o_acc, den):
                jsl = slice(JC * jc, JC * (jc + 1))
                it0, it1 = 2 * p, 2 * p + 1
                bp = psb.tile([128, 2, 512], F32, tag="beta", name="bp")
                # full-128-row scores: qq/kk are duplicated across partition
                # halves, so these compute 2*beta -- but keep the PE in
                # 128-row mode for the whole pair (no row-group switches, so
                # every LDWEIGHTS hides under the previous matmul's stream
                # and back-to-back issue stays at N/2.4GHz).
                nc.tensor.matmul(
                    bp[:, 0, :],
                    qq_sb[:, 128 * it0 : 128 * (it0 + 1)],
                    kk_sb[:, jsl],
                    start=True, stop=True,
                )
                nc.tensor.matmul(
                    bp[:, 1, :],
                    qq_sb[:, 128 * it1 : 128 * (it1 + 1)],
                    kk_sb[:, jsl],
                    start=True, stop=True,
                )
                e_t = ep.tile([128, 2, 512], F8, tag="e", name="et")
                nc.scalar.activation(
                    e_t[:, :, 0:EXP_SPLIT], bp[:, :, 0:EXP_SPLIT],
                    mybir.ActivationFunctionType.Exp,
                    scale=0.0625,
                )
                nc.vector.tensor_scalar(
                    e_t[:, :, EXP_SPLIT:512].bitcast(I8),
                    bp[:, :, EXP_SPLIT:512],
                    SCH_A, SCH_B,
                    mybir.AluOpType.mult, mybir.AluOpType.add,
                )
                first = p == 0
                last = p == N_PAIR - 1
                for cc in range(2):
                    nc.tensor.matmul(
                        o_acc[cc][:],
                        vtg[:, it0 : it0 + 2, 128 * cc : 128 * (cc + 1)],
                        e_t[:, :, :],
                        start=first, stop=last,
                        perf_mode=DR,
                    )
                nc.tensor.matmul(
                    den[0:1, :],
                    ones_f8[:, :, 0:1],
                    e_t[:, :, :],
                    start=first, stop=last,
                    perf_mode=DR,
                )

            def emit_epilogue(jc, o_acc, den):
                jsl = slice(JC * jc, JC * (jc + 1))
                # order matters at the jc boundary: free the den bank first
                # (next jc's den matmul waits on it), then one oacc copy on
                # ACT (which has a natural hole while the next jc's first
                # scores run) and one on DVE
                dsum = miscp.tile([1, 512], F32, tag="dsum")
                nc.vector.tensor_copy(dsum[:], den[0:1, :])
                oc_sb = []
                for cc in range(2):
                    t = osbp.tile([128, 512], F32, tag=f"ocp{cc}", name=f"ocp{cc}")
                    if cc == 0:
                        nc.scalar.copy(t[:], o_acc[cc][:])
                    else:
                        nc.vector.tensor_copy(t[:], o_acc[cc][:])
                    oc_sb.append(t)
                # reciprocal is expensive per free-element on DVE; remap the
                # 512 values across 128 partitions via SBUF->SBUF DMA first
                d128 = miscp.tile([128, 4], F32, tag="d128")
                nc.sync.dma_start(d128[:, :], dsum[0:1, :])
                r128 = miscp.tile([128, 4], F32, tag="r128")
                nc.vector.reciprocal(r128[:], d128[:])
                r128_bf = miscp.tile([128, 4], BF16, tag="r128bf")
                nc.vector.tensor_copy(r128_bf[:], r128[:])
                rden_bf = miscp.tile([1, 512], BF16, tag="rdenbf")
                nc.sync.dma_start(rden_bf[0:1, :], r128_bf[:, :])
                rb_ps = psmisc.tile([128, 512], F32, tag="psm", name="rbps")
                nc.tensor.matmul(
                    rb_ps[:], ones_row_bf[:], rden_bf[:], start=True, stop=True
                )
                for cc in range(2):
                    o_n = osbp.tile([128, 512], F32, tag="on")
                    nc.vector.tensor_mul(o_n[:], oc_sb[cc][:], rb_ps[:])
                    res = osbp.tile([128, 512], F32, tag="res")
                    nc.vector.scalar_tensor_tensor(
                        res[:],
                        in0=o_n[:],
                        scalar=gbv[cc][:],
                        in1=x_sb[cc][:, jsl],
                        op0=mybir.AluOpType.add,
                        op1=mybir.AluOpType.add,
                    )
                    nc.sync.dma_start(
                        out_ext.ap()[128 * cc : 128 * (cc + 1), jsl], res[:]
                    )

            def alloc_acc():
                o_acc = [
                    psacc.tile([128, 512], F32, tag=f"oacc{cc}", name=f"oacc{cc}")
                    for cc in range(2)
                ]
                den = psden.tile([128, 512], F32, tag="den", name="den")
                return o_acc, den

            # ---- prologue: all x DMAs up front; relus hoisted two chunks
            # ahead of the QKV compute so the PE never waits on the ACT/DVE
            # queues between chunks (keeps the tensor engine ramped) ----
            for n in range(N_JC):
                emit_xdma(n)
            emit_relu(0)
            emit_relu(1)
            for n in range(N_JC):
                if n + 2 < N_JC:
                    emit_relu(n + 2)
                emit_chunk(n)
            for jc in range(N_JC):
                o_acc, den = alloc_acc()
                for p in range(N_PAIR):
                    emit_pair(jc, p, o_acc, den)
                emit_epilogue(jc, o_acc, den)
    _prune_redundant_waits(nc)
    if split_waits:
        _split_multi_waits(nc)
    return nc


_NC_CACHE = None


def kernel(x, Wqkv, bqkv, gamma):
    global _NC_CACHE
    if _NC_CACHE is None:
        _NC_CACHE = build()
    nc = _NC_CACHE
    B = x.shape[0]
    assert B == N_CORES
    in_maps = []
    for i in range(B):
        in_maps.append(
            {
                "x": np.ascontiguousarray(x[i].reshape(C, HW), dtype=np.float32),
                "Wqkv": np.ascontiguousarray(Wqkv, dtype=np.float32),
                "bqkv": np.ascontiguousarray(
                    np.asarray(bqkv).reshape(2 * M + C, 1), dtype=np.float32
                ),
                "gamma": np.ascontiguousarray(
                    np.asarray(gamma).reshape(1, 1), dtype=np.float32
                ),
            }
        )
    res = run_bass_kernel_spmd(nc, in_maps, core_ids=list(range(N_CORES)))
    out = np.stack(
        [res.results[i]["out"].reshape(C, 64, 64) for i in range(N_CORES)]
    ).astype(np.float32)
    return out

